# revision 1
# baseline (speedup 1.0000x reference)
"""Binarized DenseNet (nn_DenseNet_5841155522939) Trainium2 Bass kernel.

Strategy (data-parallel over batch, 8 cores x 32 samples):
  - All post-layer-1 activations and all conv/fc weights are exactly +-1, so
    everything runs in fp8e4 with exact integer accumulation in fp32 PSUM.
  - BN+sign for layers 2..6 is replaced by per-channel integer thresholds
    (host-enumerated, bit-exact vs the f32 reference formula).
  - Convs are computed as tap-stacked matmuls on a zero-padded "canvas"
    layout [128 partitions: rows 0-63 = activations, rows 64-127 = shifted
    copies], 5 matmuls per 512-pixel chunk, two chunks running concurrently
    on PE column groups 0-1 / 2-3 (tile_position packing).
  - Layer 1 (cin=1, stride 2, f32 input) streams an exact 3-way bf16 split
    of x through a single K=27 matmul (9 taps x 3 splits), built via a
    parity-split DRAM staging roundtrip.
  - Layer 6 (stride 2) reads the canvas with stride-2 access patterns.
  - FC uses K=128 feature chunks with M=96 (12 classes x 8 pixel slots) and
    a diagonal extraction, accumulated over 64 matmuls in one PSUM bank.
"""

import numpy as np
from contextlib import ExitStack

import concourse.bacc as bacc
import concourse.bass as bass
import concourse.tile as tile
from concourse import mybir
from concourse.bass_utils import run_bass_kernel_spmd

FP8 = mybir.dt.float8e4
BF16 = mybir.dt.bfloat16
F32 = mybir.dt.float32
NP_FP8 = mybir.dt.np(FP8)
NP_BF16 = mybir.dt.np(BF16)

B = 256
NCORES = 8
BPC = B // NCORES          # 32 samples per core
GB = 4                     # samples per group
G = BPC // GB              # 8 groups
NCH = 64
WP = 66                    # padded canvas row stride (64 + 2)
S1 = WP * WP               # 4356 canvas cells per sample
S = GB * S1                # canvas cells per group
EPS = np.float32(1e-5)

# conv2-5 mm table: (rhs offset rel. to interior pixel, region, lo tap, hi tap|None)
# region0: rows0-63 = copy0, rows64-127 = copy0 shifted +132 (2 canvas rows)
# region1: rows0-63 = copy0 (dup), rows64-127 = copy0 shifted +1
MM5 = [
    (-WP - 1, 0, (0, 0), (2, 0)),
    (-WP,     0, (0, 1), (2, 1)),
    (-WP + 1, 0, (0, 2), (2, 2)),
    (-1,      1, (1, 0), (1, 1)),
    (+1,      1, (1, 2), None),
]
# conv6 (stride 2): stream base s(h',w') = (2h')*66 + 2w'; tap offset dy*66+dx
MM6 = [
    (0, 0, (0, 0), (2, 0)),
    (1, 0, (0, 1), (2, 1)),
    (2, 0, (0, 2), (2, 2)),
    (WP, 1, (1, 0), (1, 1)),
    (WP + 2, 1, (1, 2), None),
]


def _thresholds(g, b, m, v, cmax=600):
    """Per-channel (scale, bias) s.t. Sign(scale*c + bias) == reference
    sign((c-m)*g*rsqrt(v+eps)+b) for every integer c in [-cmax, cmax]."""
    inv = (np.float32(1.0) / np.sqrt((v + EPS).astype(np.float32))).astype(np.float32)
    s = (g * inv).astype(np.float32)
    C = np.arange(-cmax, cmax + 1, dtype=np.float32)
    P = len(g)
    scale = np.zeros(P, np.float32)
    bias = np.zeros(P, np.float32)
    for c in range(P):
        vals = ((C - m[c]) * s[c] + b[c]).astype(np.float32)
        sg = np.sign(vals)
        if np.any(sg == 0.0):
            raise RuntimeError(f"exact-zero BN output, channel {c}")
        if np.all(sg == sg[0]):
            scale[c] = 0.0
            bias[c] = sg[0]
            continue
        d = np.diff(sg)
        idx = np.nonzero(d)[0]
        if len(idx) != 1:
            raise RuntimeError(f"non-monotone BN sign, channel {c}")
        T = C[idx[0] + 1]
        if sg[0] < 0:
            scale[c] = 1.0
            bias[c] = np.float32(-(T - 0.5))
        else:
            scale[c] = -1.0
            bias[c] = np.float32(T - 0.5)
    return scale, bias


def _sap(t, prow, pcount, off, dims):
    """AP into tile/tensor ap `t` ([:] view): partition rows [prow, prow+pcount),
    free offset `off` elements, free dims `dims` = [[step, count], ...]."""
    ps = t.ap[0][0]
    return bass.AP(tensor=t.tensor, offset=t.offset + prow * ps + off,
                   ap=[[ps, pcount]] + dims)


def _memset_pads(nc, canvas):
    """Zero only the pad cells of region0 rows 0-63 (top/bottom rows and
    left/right columns of each sample's 66x66 canvas). Everything else is
    fully written before being read."""
    t = canvas[:]
    # top + bottom pad rows of each sample
    nc.gpsimd.memset(_sap(t, 0, 64, 0, [[S1, GB], [65 * WP, 2], [1, WP]]), 0.0)
    # left + right pad columns
    nc.gpsimd.memset(_sap(t, 0, 64, 0, [[S1, GB], [WP, WP], [65, 2]]), 0.0)


def _region_copies(nc, canvas, b, span=2):
    """Shifted-copy maintenance for a sample block ending at sample b:
    region1-lo = copy0 dup, region0-hi = copy0 shifted +132,
    region1-hi = copy0 shifted +1. span=1 gives lower tail latency
    (finer consumer overlap), span=2 fewer DMAs."""
    t = canvas[:]
    o = (b - span + 1) * S1
    L = span * S1
    nc.gpsimd.dma_start(out=_sap(t, 0, 64, S + o, [[1, L]]),
                        in_=_sap(t, 0, 64, o, [[1, L]]))
    nc.gpsimd.dma_start(out=_sap(t, 64, 64, o, [[1, L - 132]]),
                        in_=_sap(t, 0, 64, o + 132, [[1, L - 132]]))
    nc.gpsimd.dma_start(out=_sap(t, 64, 64, S + o, [[1, L - 1]]),
                        in_=_sap(t, 0, 64, o + 1, [[1, L - 1]]))


def _build_nc():
    nc = bacc.Bacc("TRN2", target_bir_lowering=False, debug=False,
                   num_devices=NCORES)
    d_x = nc.dram_tensor("xs", [BPC, 1, 128, 128], F32, kind="ExternalInput")
    d_w1 = nc.dram_tensor("w1s", [27, NCH], BF16, kind="ExternalInput")
    d_wc = nc.dram_tensor("wc", [128, 5, 5, NCH], FP8, kind="ExternalInput")
    d_sb = nc.dram_tensor("sb", [128, 2, 6], F32, kind="ExternalInput")
    d_wfc = nc.dram_tensor("wfc_l", [128, 64, 96], FP8, kind="ExternalInput")
    d_bfc = nc.dram_tensor("bfc_t", [12, 1], F32, kind="ExternalInput")
    d_out = nc.dram_tensor("out", [BPC, 12], F32, kind="ExternalOutput")

    SIGN = mybir.ActivationFunctionType.Sign

    with tile.TileContext(nc) as tc, ExitStack() as ctx:
        constp = ctx.enter_context(tc.tile_pool(name="const", bufs=1))
        canvasp = ctx.enter_context(tc.tile_pool(name="canvas", bufs=3))
        x27p = ctx.enter_context(tc.tile_pool(name="x27", bufs=2))
        frontp = ctx.enter_context(tc.tile_pool(name="front", bufs=1))
        stagep = ctx.enter_context(tc.tile_pool(name="stage", bufs=3))
        psump = ctx.enter_context(tc.tile_pool(name="psum", bufs=2, space="PSUM"))
        dramp = ctx.enter_context(tc.tile_pool(name="dram", bufs=2, space="DRAM"))

        # ---- constants
        w1 = constp.tile([27, NCH], BF16)
        wc = constp.tile([128, 5, 5, NCH], FP8)
        sb = constp.tile([128, 2, 6], F32)
        wfc = constp.tile([128, 64, 96], FP8)
        bfc = constp.tile([12, 1], F32)
        act6 = constp.tile([128, BPC * 1024], FP8)
        first_canvases = []
        first_x27 = []
        first_stg = []

        def emit_front(g):
            """Layer-1 input pipeline for group g: load x, 3-way bf16 split,
            parity rearrange, DRAM staging, X27 gathers. Returns X27 halves."""
            X = frontp.tile([128, GB * 128], F32, tag="X")
            # x[g*GB + b, 0, h, w] -> X[h, (b, w)]
            nc.sync.dma_start(out=X[:], in_=bass.AP(
                tensor=d_x[:].tensor, offset=g * GB * 16384,
                ap=[[128, 128], [16384, GB], [1, 128]]))
            s0u = frontp.tile([128, GB * 128], BF16, tag="s0u")
            s1u = frontp.tile([128, GB * 128], BF16, tag="s1u")
            s2u = frontp.tile([128, GB * 128], BF16, tag="s2u")
            r1 = frontp.tile([128, GB * 128], F32, tag="r1")
            r2 = frontp.tile([128, GB * 128], F32, tag="r2")
            nc.vector.tensor_copy(s0u[:], X[:])
            nc.vector.tensor_sub(r1[:], X[:], s0u[:])
            nc.vector.tensor_copy(s1u[:], r1[:])
            nc.vector.tensor_sub(r2[:], r1[:], s1u[:])
            nc.vector.tensor_copy(s2u[:], r2[:])
            # parity-w rearrange: SP[h, (s, b, px, w')] bf16
            SP = frontp.tile([128, 3, GB, 2, 64], BF16, tag="SP")
            for si, st in enumerate((s0u, s1u, s2u)):
                for px in range(2):
                    nc.vector.tensor_copy(
                        SP[:, si, :, px, :],
                        bass.AP(tensor=st[:].tensor, offset=st[:].offset + px,
                                ap=[[GB * 128, 128], [128, GB], [2, 64]]))
            # stage to DRAM parity-row-split: flat [py][s][b][px][h'][w']
            # (h' adjacent to w' so the X27 gathers read fat contiguous runs)
            FRO = 3 * GB * 2 * 64            # 1536: SP free elems per partition
            DS = dramp.tile([2 * 64 * FRO], BF16)
            dsap = DS[:]
            for py in range(2):
                src = bass.AP(tensor=SP[:].tensor,
                              offset=SP[:].offset + py * FRO,
                              ap=[[2 * FRO, 64], [1, FRO]])
                dst = bass.AP(tensor=dsap.tensor,
                              offset=dsap.offset + py * (3 * GB * 2 * 4096),
                              ap=[[64, 64], [4096, 24], [1, 64]])
                nc.sync.dma_start(out=dst, in_=src)
            halves = []
            for hb in range(2):
                X27 = x27p.tile([32, 2 * 4096], BF16, tag="x27")
                if len(first_x27) < 2:
                    first_x27.append(X27)
                    nc.gpsimd.memset(X27[:], 0.0)
                for t9 in range(9):
                    dy, dx = t9 // 3, t9 % 3
                    pyy, rh = (dy - 1) % 2, (dy - 1 - (dy - 1) % 2) // 2
                    pxx, rw = (dx - 1) % 2, (dx - 1 - (dx - 1) % 2) // 2
                    h0, w0 = -rh, -rw          # dst start (0 or 1)
                    cnt_h, cnt_w = 64 - h0, 64 - w0
                    for bb in range(2):
                        # one DMA per (tap, sample): all 3 splits land on
                        # X27 partitions 3t..3t+2 (DS s-stride GB*2*4096)
                        soff = (dsap.offset + pyy * (3 * GB * 2 * 4096)
                                + pxx * 4096 + (h0 + rh) * 64 + (w0 + rw)
                                + (hb * 2 + bb) * 2 * 4096)
                        src = bass.AP(tensor=dsap.tensor, offset=soff,
                                      ap=[[GB * 2 * 4096, 3],
                                          [64, cnt_h], [1, cnt_w]])
                        dst = _sap(X27[:], 3 * t9, 3,
                                   bb * 4096 + h0 * 64 + w0,
                                   [[64, cnt_h], [1, cnt_w]])
                        nc.sync.dma_start(out=dst, in_=src)
                halves.append(X27)
            return halves

        front = emit_front(0)
        nc.sync.dma_start(out=w1[:], in_=d_w1[:])
        nc.sync.dma_start(out=wc[:], in_=d_wc[:])
        nc.sync.dma_start(out=sb[:], in_=d_sb[:])
        nc.sync.dma_start(out=wfc[:], in_=d_wfc[:])
        nc.sync.dma_start(out=bfc[:], in_=d_bfc[:])
        for g in range(G):
            # ================= conv1 matmuls -> L1 canvas =================
            x27_halves = front
            cur = canvasp.tile([128, 2 * S], FP8, tag="canvas")
            if len(first_canvases) < 3:
                first_canvases.append(cur)
                _memset_pads(nc, cur)
            for hb in range(2):        # half-groups of 2 samples
                X27 = x27_halves[hb]
                for b2 in range(2):
                    b = 2 * hb + b2
                    ps = psump.tile([128, 2048], F32, tag="ps")
                    stg = stagep.tile([128, 4, 8 * WP], FP8, tag="stg")
                    if len(first_stg) < 3:
                        first_stg.append(stg)
                        nc.gpsimd.memset(stg[:], 0.0)
                    for q in range(4):
                        for half in range(2):
                            h0 = 16 * q + 8 * half
                            rhs = _sap(X27[:], 0, 27, b2 * 4096 + h0 * 64,
                                       [[64, 8], [1, 64]])
                            nc.tensor.matmul(
                                ps[64 * half:64 * half + 64, 512 * q:512 * q + 512],
                                lhsT=w1[:], rhs=rhs, start=True, stop=True)
                    # staging has the canvas 66-stride built in (gaps stay 0)
                    nc.scalar.activation(
                        _sap(stg[:], 0, 128, 1, [[8 * WP, 4], [WP, 8], [1, 64]]),
                        _sap(ps[:], 0, 128, 0, [[512, 4], [64, 8], [1, 64]]),
                        SIGN, bias=sb[:, 1, 0:1], scale=sb[:, 0, 0:1])
                    for half in range(2):
                        src = _sap(stg[:], 64 * half, 64, 0,
                                   [[8 * WP, 4], [1, 8 * WP]])
                        dst = _sap(cur[:], 0, 64, b * S1 + WP + 8 * WP * half,
                                   [[16 * WP, 4], [1, 8 * WP]])
                        nc.sync.dma_start(out=dst, in_=src)
                    if b % 2 == 1:
                        _region_copies(nc, cur, b)

            # ================= conv2..conv5 =================
            for li in range(4):          # wc layer index 0..3, sb index 1..4
                if li == 0 and g + 1 < G:
                    # prefetch next group's layer-1 pipeline mid-group
                    front = emit_front(g + 1)
                nxt = canvasp.tile([128, 2 * S], FP8, tag="canvas")
                if len(first_canvases) < 3:
                    first_canvases.append(nxt)
                    _memset_pads(nc, nxt)
                for b in range(GB):
                    ps = psump.tile([128, 2048], F32, tag="ps")
                    stg = stagep.tile([128, 4, 8 * WP], FP8, tag="stg")
                    if len(first_stg) < 3:
                        first_stg.append(stg)
                        nc.gpsimd.memset(stg[:], 0.0)
                    for q in range(4):
                        for half in range(2):
                            h0 = 16 * q + 8 * half
                            ibase = b * S1 + (h0 + 1) * WP + 1
                            for i, (o, reg, _, _) in enumerate(MM5):
                                rhs = _sap(cur[:], 0, 128, reg * S + ibase + o,
                                           [[WP, 8], [1, 64]])
                                nc.tensor.matmul(
                                    ps[64 * half:64 * half + 64,
                                       512 * q:512 * q + 512],
                                    lhsT=wc[:, li, i, :], rhs=rhs,
                                    start=(i == 0), stop=(i == 4))
                    nc.scalar.activation(
                        _sap(stg[:], 0, 128, 1, [[8 * WP, 4], [WP, 8], [1, 64]]),
                        _sap(ps[:], 0, 128, 0, [[512, 4], [64, 8], [1, 64]]),
                        SIGN, bias=sb[:, 1, li + 1:li + 2],
                        scale=sb[:, 0, li + 1:li + 2])
                    for half in range(2):
                        src = _sap(stg[:], 64 * half, 64, 0,
                                   [[8 * WP, 4], [1, 8 * WP]])
                        dst = _sap(nxt[:], 0, 64, b * S1 + WP + 8 * WP * half,
                                   [[16 * WP, 4], [1, 8 * WP]])
                        nc.sync.dma_start(out=dst, in_=src)
                    if b % 2 == 1:
                        _region_copies(nc, nxt, b)
                cur = nxt

            # ================= conv6 (stride 2) -> act6 =================
            ps = psump.tile([128, 2048], F32, tag="ps")
            stg = stagep.tile([128, 2048], FP8, tag="stg6")
            for b in range(GB):
                for half in range(2):
                    h0 = 16 * half        # output rows [h0, h0+16)
                    sbase = b * S1 + (2 * h0) * WP
                    for i, (o, reg, _, _) in enumerate(MM6):
                        rhs = _sap(cur[:], 0, 128, reg * S + sbase + o,
                                   [[2 * WP, 16], [2, 32]])
                        nc.tensor.matmul(
                            ps[64 * half:64 * half + 64, 512 * b:512 * b + 512],
                            lhsT=wc[:, 4, i, :], rhs=rhs,
                            start=(i == 0), stop=(i == 4))
            nc.scalar.activation(stg[:], ps[:], SIGN,
                                 bias=sb[:, 1, 5:6], scale=sb[:, 0, 5:6])
            for half in range(2):
                src = _sap(stg[:], 64 * half, 64, 0, [[512, 4], [1, 512]])
                dst = _sap(act6[:], 0, 64, (g * GB) * 1024 + 512 * half,
                           [[1024, 4], [1, 512]])
                nc.sync.dma_start(out=dst, in_=src)
            # upper fc operand half for this group (act6 shifted +512 per sample)
            nc.gpsimd.dma_start(
                out=_sap(act6[:], 64, 64, g * GB * 1024, [[1024, GB], [1, 512]]),
                in_=_sap(act6[:], 0, 64, g * GB * 1024 + 512, [[1024, GB], [1, 512]]))

        # ================= fc =================
        # 512 accumulating matmuls: K=128 features (ch x {p, p+512}), M=12, N=32
        psf = psump.tile([12, BPC], F32, tag="ps")
        for p in range(512):
            rhs = _sap(act6[:], 0, 128, p, [[1024, BPC]])
            nc.tensor.matmul(psf[:], lhsT=wfc[:, p // 8, 12 * (p % 8):12 * (p % 8) + 12],
                             rhs=rhs, start=(p == 0), stop=(p == 511))
        accf = constp.tile([12, BPC], F32)
        nc.vector.tensor_scalar_add(accf[:], psf[:], bfc[:])
        nc.sync.dma_start(
            out=bass.AP(tensor=d_out[:].tensor, offset=0,
                        ap=[[1, 12], [12, BPC]]),
            in_=accf[:])

    nc.compile()
    return nc


_NC_CACHE = {}


def _prep_const_inputs(inputs):
    out = {}
    # layer-1 weights: [27 = 3*t + s, cout] bf16 (same tap weight for each split)
    w1b = np.sign(np.asarray(inputs["w1"], np.float32))  # [64, 1, 3, 3]
    w1s = np.zeros((27, NCH), NP_BF16)
    for t9 in range(9):
        dy, dx = t9 // 3, t9 % 3
        for s3 in range(3):
            w1s[3 * t9 + s3, :] = w1b[:, 0, dy, dx].astype(NP_BF16)
    out["w1s"] = w1s
    # conv2-6 mm weights
    wc = np.zeros((128, 5, 5, NCH), NP_FP8)
    for li in range(5):
        w = np.sign(np.asarray(inputs[f"w{li + 2}"], np.float32))  # [O, I, 3, 3]
        table = MM5 if li < 4 else MM6
        for i, (_, _, lo, hi) in enumerate(table):
            wc[0:64, li, i, :] = w[:, :, lo[0], lo[1]].T.astype(NP_FP8)
            if hi is not None:
                wc[64:128, li, i, :] = w[:, :, hi[0], hi[1]].T.astype(NP_FP8)
    out["wc"] = wc
    # scales/biases [128, 2, 6]
    sb = np.zeros((128, 2, 6), np.float32)
    g1, b1, m1, v1 = (np.asarray(inputs[k], np.float32) for k in
                      ("g1", "b1", "m1", "v1"))
    inv = (np.float32(1.0) / np.sqrt((v1 + EPS).astype(np.float32))).astype(np.float32)
    s1 = (g1 * inv).astype(np.float32)
    sb[:, 0, 0] = np.tile(s1, 2)
    sb[:, 1, 0] = np.tile((b1 - m1 * s1).astype(np.float32), 2)
    for li in range(1, 6):
        g_, b_, m_, v_ = (np.asarray(inputs[f"{k}{li + 1}"], np.float32)
                          for k in ("g", "b", "m", "v"))
        sc, bi = _thresholds(g_, b_, m_, v_)
        sb[:, 0, li] = np.tile(sc, 2)
        sb[:, 1, li] = np.tile(bi, 2)
    out["sb"] = sb
    # fc: lhsT [128, chunk k, m = j*12 + cls]
    wfc = np.sign(np.asarray(inputs["wfc"], np.float32)).reshape(12, 64, 1024)
    wl = np.zeros((128, 64, 96), NP_FP8)
    for k in range(64):
        for j in range(8):
            p = 8 * k + j
            wl[0:64, k, 12 * j:12 * j + 12] = wfc[:, :, p].T.astype(NP_FP8)
            wl[64:128, k, 12 * j:12 * j + 12] = wfc[:, :, p + 512].T.astype(NP_FP8)
    out["wfc_l"] = wl
    out["bfc_t"] = np.asarray(inputs["bfc"], np.float32).reshape(12, 1)
    return out


def kernel(**inputs):
    if "nc" not in _NC_CACHE:
        _NC_CACHE["nc"] = _build_nc()
    nc = _NC_CACHE["nc"]
    const = _prep_const_inputs(inputs)
    x = np.asarray(inputs["x"], np.float32)
    in_maps = []
    for c in range(NCORES):
        m = dict(const)
        m["xs"] = np.ascontiguousarray(x[c * BPC:(c + 1) * BPC])
        in_maps.append(m)
    res = run_bass_kernel_spmd(nc, in_maps, core_ids=list(range(NCORES)))
    return np.concatenate([r["out"] for r in res.results], axis=0)



# revision 26
# speedup vs baseline: 2.5508x; 2.5508x over previous
"""Binarized DenseNet (nn_DenseNet_5841155522939) Trainium2 Bass kernel.

Strategy (data-parallel, 8 cores x 32 samples, processed as 16 sample-PAIRS):
  - Post-layer-1 activations and conv weights are exactly +-1: everything runs
    in fp8e4 with exact integer accumulation in fp32 PSUM.
  - Canvas layout per pair: [128 partitions = 2 samples x 64 ch, 66x66 cells]
    with a full duplicate at free offset 4367, so a +1-column tap partner sits
    at pair-stride 4368 (16B-aligned, required by DoubleRow).
  - conv2..conv6 use fp8 DoubleRow matmuls (2 taps per pair dim, 0.5 cyc/row):
    5 matmuls per 512-pixel chunk cover all 9 taps for BOTH samples (M=128
    block-diagonal), K_eff = 256.
  - BN+sign becomes per-channel (scale, bias) on the integer conv sums;
    applied by ScalarE Sign activations writing fp8 straight into the next
    canvas (no staging DMAs). One half-layer plus conv6 run on VectorE
    (mult -> is_ge) to balance engine load.
  - conv6 output is stored as {0,1} (single is_ge op); the FC layer absorbs
    the 2u-1 decode into a folded bias and a final x2 scale.
  - Layer 1 (cin=1, stride 2, f32 input): exact 3-way bf16 split of x, one
    K=54 matmul per chunk (9 taps x 3 splits x 2 samples, block-diagonal),
    staged via a parity-split DRAM roundtrip (one gather DMA per tap).
  - FC: 256 DoubleRow matmuls, K=256 features each, N=32 samples.
"""

import numpy as np
from contextlib import ExitStack

import concourse.bacc as bacc
import concourse.bass as bass
import concourse.tile as tile
from concourse import mybir
from concourse.bass_utils import run_bass_kernel_spmd

FP8 = mybir.dt.float8e4
BF16 = mybir.dt.bfloat16
F32 = mybir.dt.float32
NP_FP8 = mybir.dt.np(FP8)
NP_BF16 = mybir.dt.np(BF16)
DR = mybir.MatmulPerfMode.DoubleRow
ALU = mybir.AluOpType
SIGN = mybir.ActivationFunctionType.Sign

B = 256
NCORES = 8
BPC = B // NCORES          # 32 samples per core
NPAIR = BPC // 2           # 16 pairs per core
NCH = 64
WP = 66                    # canvas row pitch
S1 = WP * WP               # 4356 cells per canvas
DUPO = 4367                # duplicate canvas base (DUPO % 16 == 15)
P1 = DUPO + 1              # pair stride hitting canvas[o+1] (4368, 16-aligned)
CSZ = 8736                 # canvas tile free size
EPS = np.float32(1e-5)

# conv2-5 matmuls: (offset rel. to interior pixel p, pair stride, lo tap, hi tap)
# tap (r, c) = kernel index; cell = p + (r-1)*66 + (c-1)
MM5 = [
    (-67, P1, (0, 0), (0, 1)),
    (-65, 64, (0, 2), (1, 0)),
    (0,   P1, (1, 1), (1, 2)),
    (65,  P1, (2, 0), (2, 1)),
    (67,  64, (2, 2), None),
]
# conv6 (stride 2): base b6 = 2h'*66 + 2w'; tap (r, c) at b6 + r*66 + c
MM6 = [
    (0,   P1, (0, 0), (0, 1)),
    (2,   64, (0, 2), (1, 0)),
    (67,  P1, (1, 1), (1, 2)),
    (132, P1, (2, 0), (2, 1)),
    (134, 64, (2, 2), None),
]

# (layer li in 0..4 = conv1..conv5, psum half qh) sign ops done on VectorE
# instead of ScalarE, to balance engine load. conv6 is always on VectorE.
DVE_SIGNS = {(1, 1), (2, 1)}


def _thresholds(g, b, m, v, cmax=600):
    """Per-channel (scale, bias) s.t. Sign(scale*c + bias) == reference
    sign((c-m)*g*rsqrt(v+eps)+b) for every integer c in [-cmax, cmax]."""
    inv = (np.float32(1.0) / np.sqrt((v + EPS).astype(np.float32))).astype(np.float32)
    s = (g * inv).astype(np.float32)
    C = np.arange(-cmax, cmax + 1, dtype=np.float32)
    P = len(g)
    scale = np.zeros(P, np.float32)
    bias = np.zeros(P, np.float32)
    for c in range(P):
        vals = ((C - m[c]) * s[c] + b[c]).astype(np.float32)
        sg = np.sign(vals)
        if np.any(sg == 0.0):
            raise RuntimeError(f"exact-zero BN output, channel {c}")
        if np.all(sg == sg[0]):
            scale[c] = 0.0
            bias[c] = sg[0]
            continue
        d = np.diff(sg)
        idx = np.nonzero(d)[0]
        if len(idx) != 1:
            raise RuntimeError(f"non-monotone BN sign, channel {c}")
        T = C[idx[0] + 1]
        if sg[0] < 0:
            scale[c] = 1.0
            bias[c] = np.float32(-(T - 0.5))
        else:
            scale[c] = -1.0
            bias[c] = np.float32(T - 0.5)
    return scale, bias


def _sap(t, prow, pcount, off, dims):
    """AP into tile view `t` ([:] view): partition rows [prow, prow+pcount),
    free offset `off` elements, free dims [[step, count], ...]."""
    ps = t.ap[0][0]
    return bass.AP(tensor=t.tensor, offset=t.offset + prow * ps + off,
                   ap=[[ps, pcount]] + dims)


def _build_nc():
    nc = bacc.Bacc("TRN2", target_bir_lowering=False, debug=False,
                   num_devices=NCORES)
    d_x = nc.dram_tensor("xs", [BPC, 1, 128, 128], F32, kind="ExternalInput")
    d_w1 = nc.dram_tensor("w1s", [64, 128], BF16, kind="ExternalInput")
    d_wc = nc.dram_tensor("wc", [128, 5, 5, 2, 128], FP8, kind="ExternalInput")
    d_sb = nc.dram_tensor("sb", [128, 2, 6], F32, kind="ExternalInput")
    d_sb2 = nc.dram_tensor("sb2", [128, 2, 6], F32, kind="ExternalInput")
    d_wfc = nc.dram_tensor("wfc_l", [128, 256, 2, 16], FP8, kind="ExternalInput")
    d_bfc = nc.dram_tensor("bfc_t", [12, 1], F32, kind="ExternalInput")
    d_z = nc.dram_tensor("zrow", [5, 4096], BF16, kind="ExternalInput")
    d_out = nc.dram_tensor("out", [BPC, 12], F32, kind="ExternalOutput")

    with tile.TileContext(nc) as tc, ExitStack() as ctx:
        constp = ctx.enter_context(tc.tile_pool(name="const", bufs=1))
        canvasp = ctx.enter_context(tc.tile_pool(name="canvas", bufs=8))
        x27p = ctx.enter_context(tc.tile_pool(name="x27", bufs=4))
        frontp = ctx.enter_context(tc.tile_pool(name="front", bufs=2))
        tmpp = ctx.enter_context(tc.tile_pool(name="tmp", bufs=3))
        psump = ctx.enter_context(tc.tile_pool(name="psum", bufs=2, space="PSUM"))
        dramp = ctx.enter_context(tc.tile_pool(name="dram", bufs=2, space="DRAM"))

        # ---- constants
        w1 = constp.tile([64, 128], BF16)
        wc = constp.tile([128, 5, 5, 2, 128], FP8)
        sb = constp.tile([128, 2, 6], F32)
        sb2 = constp.tile([128, 2, 6], F32)
        wfc = constp.tile([128, 256, 2, 16], FP8)
        bfc = constp.tile([12, 1], F32)
        act6 = constp.tile([128, BPC * 512], FP8)

        def new_canvas():
            cnv = canvasp.tile([128, CSZ], FP8, tag="canvas")
            t = cnv[:]
            # pad rows/cols of the main canvas; gap + tail cells
            # (top row; bottom row + gap; tail; side cols)
            nc.gpsimd.memset(_sap(t, 0, 128, 0, [[1, WP]]), 0.0)
            nc.gpsimd.memset(_sap(t, 0, 128, 65 * WP, [[1, DUPO - 65 * WP]]), 0.0)
            nc.gpsimd.memset(_sap(t, 0, 128, DUPO + S1, [[1, CSZ - DUPO - S1]]), 0.0)
            nc.gpsimd.memset(_sap(t, 0, 128, 0, [[WP, WP], [65, 2]]), 0.0)
            return cnv

        HWP = 33 * WP    # dup half size (rows 0-32 / 33-65)

        def sign_to_canvas(ps, cnv, qh, li):
            """BN+sign psum[128,2048] (rows 32qh..32qh+32) -> canvas interior,
            then copy this half (incl. pads) into the dup region. The dup DMA
            is issued by the same engine that ran the sign, so it issues with
            its dependency already satisfied (no SEQ head-of-line blocking)."""
            in_ = _sap(ps[:], 0, 128, 0, [[64, 32], [1, 64]])
            out = _sap(cnv[:], 0, 128, (32 * qh + 1) * WP + 1, [[WP, 32], [1, 64]])
            if (li, qh) in DVE_SIGNS:
                tmp = tmpp.tile([128, 2048], BF16, tag="sgtmp")
                tin = _sap(tmp[:], 0, 128, 0, [[64, 32], [1, 64]])
                nc.vector.tensor_scalar(
                    tin, in_, sb2[:, 0, li:li + 1], sb2[:, 1, li:li + 1],
                    ALU.mult, ALU.is_ge)
                nc.vector.tensor_scalar(out, tin, 2.0, -1.0, ALU.mult, ALU.add)
                eng = nc.gpsimd
            else:
                nc.scalar.activation(out, in_, SIGN,
                                     bias=sb[:, 1, li:li + 1],
                                     scale=sb[:, 0, li:li + 1])
                eng = nc.scalar
            pending_dups[-1].append((eng, cnv, qh))

        def emit_front(pp):
            """Layer-1 input pipeline for pair pp: load x, 3-way bf16 split,
            parity rearrange, DRAM staging, 9 tap-gather DMAs -> X27."""
            X = frontp.tile([128, 256], F32, tag="X")
            nc.sync.dma_start(out=X[:], in_=bass.AP(
                tensor=d_x[:].tensor, offset=pp * 2 * 16384,
                ap=[[128, 128], [16384, 2], [1, 128]]))
            s0u = frontp.tile([128, 256], BF16, tag="s0u")
            s1u = frontp.tile([128, 256], BF16, tag="s1u")
            s2u = frontp.tile([128, 256], BF16, tag="s2u")
            r1 = frontp.tile([128, 256], F32, tag="r1")
            r2 = frontp.tile([128, 256], F32, tag="r2")
            nc.vector.tensor_copy(s0u[:], X[:])
            nc.vector.tensor_sub(r1[:], X[:], s0u[:])
            nc.vector.tensor_copy(s1u[:], r1[:])
            nc.vector.tensor_sub(r2[:], r1[:], s1u[:])
            nc.vector.tensor_copy(s2u[:], r2[:])
            # parity-w rearrange, b-major: SP[h, (b, s, px, w')]
            SP = frontp.tile([128, 2, 3, 2, 64], BF16, tag="SP")
            for si, st in enumerate((s0u, s1u, s2u)):
                for px in range(2):
                    src = bass.AP(tensor=st[:].tensor, offset=st[:].offset + px,
                                  ap=[[st[:].ap[0][0], 128], [128, 2], [2, 64]])
                    nc.vector.tensor_copy(SP[:, :, si, px, :], src)
            # stage to DRAM: DS[h][b][s][px][w'] — one contiguous DMA
            DS = dramp.tile([128 * 768], BF16, tag="DS")
            dsap = DS[:]
            spv = SP[:]
            nc.sync.dma_start(
                out=bass.AP(tensor=dsap.tensor, offset=dsap.offset,
                            ap=[[768, 128], [1, 768]]),
                in_=bass.AP(tensor=spv.tensor, offset=spv.offset,
                            ap=[[spv.ap[0][0], 128], [1, 768]]))
            X27 = x27p.tile([64, 4096], BF16, tag="x27")
            # cells not written by gathers: h'=0 row (dy=0 taps), w'=0 col
            # (dx=0), and rows 27-31 (read by the K=59 matmul, zero weights)
            nc.gpsimd.memset(_sap(X27[:], 0, 64, 0, [[1, 64]]), 0.0)
            nc.gpsimd.memset(_sap(X27[:], 0, 64, 0, [[64, 64], [1, 1]]), 0.0)
            nc.scalar.dma_start(out=_sap(X27[:], 27, 5, 0, [[1, 4096]]),
                                in_=d_z[:])
            for t9 in range(9):
                dy, dx = t9 // 3, t9 % 3
                pxx, rw = (dx - 1) % 2, (dx - 1 - (dx - 1) % 2) // 2
                h0 = 1 if dy == 0 else 0
                w0 = -rw
                ch, cw = 64 - h0, 64 - w0
                r0 = 2 * h0 + dy - 1   # first input row used; stride 2 rows
                src = bass.AP(tensor=dsap.tensor,
                              offset=dsap.offset + r0 * 768 + pxx * 64,
                              ap=[[128, 3], [384, 2], [1536, ch], [1, cw]])
                dst = _sap(X27[:], 3 * t9, 3, h0 * 64 + w0,
                           [[4096, 2], [64, ch], [1, cw]])
                geng = (nc.sync, nc.sync, nc.gpsimd)[t9 % 3]
                geng.dma_start(out=dst, in_=src)
            return X27

        pending_dups = []

        def flush_dups(all_=False):
            keep = 0 if all_ else 2
            while len(pending_dups) > keep:
                for eng, cnv, qh in pending_dups.pop(0):
                    t = cnv[:]
                    eng.dma_start(
                        out=_sap(t, 0, 128, DUPO + qh * HWP, [[1, HWP]]),
                        in_=_sap(t, 0, 128, qh * HWP, [[1, HWP]]))
            pending_dups.append([])

        def conv1(X27, cur):
            flush_dups()
            for qh in range(2):
                ps = psump.tile([128, 2048], F32, tag="ps")
                for qq in range(4):
                    q = 4 * qh + qq
                    rhs = _sap(X27[:], 0, 59, q * 512, [[64, 8], [1, 64]])
                    nc.tensor.matmul(ps[:, 512 * qq:512 * qq + 512],
                                     lhsT=_sap(w1[:], 0, 59, 0, [[1, 128]]),
                                     rhs=rhs, start=True, stop=True)
                sign_to_canvas(ps, cur, qh, 0)

        def conv_mid(li, cur, nxt):
            flush_dups()
            for qh in range(2):
                ps = psump.tile([128, 2048], F32, tag="ps")
                for qq in range(4):
                    q = 4 * qh + qq
                    base = (8 * q + 1) * WP + 1
                    for i, (o, P, _, _) in enumerate(MM5):
                        rhs = _sap(cur[:], 0, 128, base + o,
                                   [[P, 2], [WP, 8], [1, 64]])
                        nc.tensor.matmul(
                            ps[:, 512 * qq:512 * qq + 512],
                            lhsT=wc[:, li - 1, i], rhs=rhs,
                            start=(i == 0), stop=(i == 4), perf_mode=DR)
                sign_to_canvas(ps, nxt, qh, li)

        def conv6(cur, pp, last=False):
            flush_dups()
            ps = psump.tile([128, 2048], F32, tag="ps")
            for h in range(2):
                for i, (o, P, _, _) in enumerate(MM6):
                    rhs = _sap(cur[:], 0, 128, h * 2112 + o,
                               [[P, 2], [132, 16], [2, 32]])
                    nc.tensor.matmul(ps[:, 512 * h:512 * h + 512],
                                     lhsT=wc[:, 4, i], rhs=rhs,
                                     start=(i == 0), stop=(i == 4), perf_mode=DR)
            stg6 = tmpp.tile([128, 1024], FP8, tag="stg6")
            nc.vector.tensor_scalar(stg6[:], ps[:, 0:1024],
                                    sb2[:, 0, 5:6], sb2[:, 1, 5:6],
                                    ALU.mult, ALU.is_ge)
            # rearrange [2s x 64ch, 1024px] -> act6[64ch x 2px-half, sample*512]
            for s in range(2):
                for ph in range(2):
                    dst = _sap(act6[:], 64 * ph, 64, (2 * pp + s) * 512,
                               [[1, 512]])
                    srcp = _sap(stg6[:], 64 * s, 64, 512 * ph, [[1, 512]])
                    eng = (nc.sync if s == 0 else nc.scalar) if last \
                        else nc.gpsimd
                    eng.dma_start(out=dst, in_=srcp)

        # Pairs interleaved layer-by-layer in groups of GIL: while one
        # pair's sign + dup-copy latency drains, the PE runs the other
        # pairs' matmuls (PE executes strictly in program order).
        GIL = 4
        fronts = {0: emit_front(0)}
        nc.scalar.dma_start(out=w1[:], in_=d_w1[:])
        nc.scalar.dma_start(out=sb[:], in_=d_sb[:])
        nc.scalar.dma_start(out=sb2[:], in_=d_sb2[:])
        nc.scalar.dma_start(out=bfc[:], in_=d_bfc[:])
        for p in range(1, min(GIL, NPAIR)):
            fronts[p] = emit_front(p)
        nc.sync.dma_start(out=wc[:], in_=d_wc[:])
        nc.sync.dma_start(out=wfc[:], in_=d_wfc[:])
        for g in range(0, NPAIR, GIL):
            grp = [g + i for i in range(GIL)]
            Xs = [fronts.pop(p) for p in grp]
            curs = []
            for idx, p in enumerate(grp):
                cur = new_canvas()
                conv1(Xs[idx], cur)
                curs.append(cur)
            for li in range(1, 5):
                for idx, p in enumerate(grp):
                    if li == 1 + idx and p + GIL < NPAIR:
                        fronts[p + GIL] = emit_front(p + GIL)
                    nxt = new_canvas()
                    conv_mid(li, curs[idx], nxt)
                    curs[idx] = nxt
            for idx, p in enumerate(grp):
                conv6(curs[idx], p, last=(g + GIL >= NPAIR))

        flush_dups(all_=True)

        # ---- fc: 256 DoubleRow matmuls, K=256 features, N=32 samples
        psf = psump.tile([12, 32], F32, tag="ps")
        for j in range(256):
            rhs = _sap(act6[:], 0, 128, j, [[256, 2], [512, 32]])
            lhsT = bass.AP(tensor=wfc[:].tensor,
                           offset=wfc[:].offset + j * 32,
                           ap=[[wfc[:].ap[0][0], 128], [16, 2], [1, 12]])
            nc.tensor.matmul(psf[:], lhsT=lhsT, rhs=rhs,
                             start=(j == 0), stop=(j == 255), perf_mode=DR)
        accf = constp.tile([12, 32], F32)
        nc.vector.tensor_scalar(accf[:], psf[:], 2.0, bfc[:],
                                ALU.mult, ALU.add)
        nc.sync.dma_start(
            out=bass.AP(tensor=d_out[:].tensor, offset=0,
                        ap=[[1, 12], [12, BPC]]),
            in_=accf[:])

    nc.compile()
    return nc


_NC_CACHE = {}


def _prep_const_inputs(inputs):
    out = {}
    # conv1 weights: [64, 128] bf16, block-diag: sample A rows 0-26 (PE row
    # group 0), sample B rows 32-58 (row group 1) -> per-sample sums use the
    # same 32-row accumulation grouping as a K=27 matmul
    w1b = np.sign(np.asarray(inputs["w1"], np.float32))  # [64, 1, 3, 3]
    w1s = np.zeros((64, 128), NP_BF16)
    for t9 in range(9):
        dy, dx = t9 // 3, t9 % 3
        for bb in range(2):
            for s3 in range(3):
                w1s[32 * bb + 3 * t9 + s3, 64 * bb:64 * bb + 64] = \
                    w1b[:, 0, dy, dx].astype(NP_BF16)
    out["w1s"] = w1s
    out["zrow"] = np.zeros((5, 4096), NP_BF16)
    # conv2-6 DoubleRow weights, block-diagonal over the 2 samples
    wcx = np.zeros((128, 5, 5, 2, 128), NP_FP8)
    for li in range(5):
        w = np.sign(np.asarray(inputs[f"w{li + 2}"], np.float32))  # [O, I, 3, 3]
        table = MM5 if li < 4 else MM6
        for i, (_, _, lo, hi) in enumerate(table):
            for j, tap in enumerate((lo, hi)):
                if tap is None:
                    continue
                blk = w[:, :, tap[0], tap[1]].T.astype(NP_FP8)  # [I, O]
                wcx[0:64, li, i, j, 0:64] = blk
                wcx[64:128, li, i, j, 64:128] = blk
    out["wc"] = wcx
    # scales/biases [128, 2, 6]: Sign form (sb) and is_ge form (sb2)
    sb = np.zeros((128, 2, 6), np.float32)
    sb2 = np.zeros((128, 2, 6), np.float32)
    g1, b1, m1, v1 = (np.asarray(inputs[k], np.float32) for k in
                      ("g1", "b1", "m1", "v1"))
    inv = (np.float32(1.0) / np.sqrt((v1 + EPS).astype(np.float32))).astype(np.float32)
    s1 = (g1 * inv).astype(np.float32)
    sb[:, 0, 0] = np.tile(s1, 2)
    sb[:, 1, 0] = np.tile((b1 - m1 * s1).astype(np.float32), 2)
    sb2[:, 0, 0] = sb[:, 0, 0]
    sb2[:, 1, 0] = -sb[:, 1, 0]
    for li in range(1, 6):
        g_, b_, m_, v_ = (np.asarray(inputs[f"{k}{li + 1}"], np.float32)
                          for k in ("g", "b", "m", "v"))
        sc, bi = _thresholds(g_, b_, m_, v_)
        sb[:, 0, li] = np.tile(sc, 2)
        sb[:, 1, li] = np.tile(bi, 2)
        sb2[:, 0, li] = np.tile(sc, 2)
        sb2[:, 1, li] = np.tile(-bi, 2)
    out["sb"] = sb
    out["sb2"] = sb2
    # fc: lhsT [128 = ch + 64*ph, j, pair i, cls(12, padded 16)]
    wfc_s = np.sign(np.asarray(inputs["wfc"], np.float32))  # [12, 65536]
    wr = wfc_s.reshape(12, 64, 2, 2, 256)  # [cls, ch, ph, i, j]
    wl = np.zeros((128, 256, 2, 16), NP_FP8)
    for ph in range(2):
        # wl[ch + 64*ph, j, i, cls]
        wl[64 * ph:64 * ph + 64, :, :, 0:12] = \
            wr[:, :, ph, :, :].transpose(1, 3, 2, 0).astype(NP_FP8)
    out["wfc_l"] = wl
    bfc = np.asarray(inputs["bfc"], np.float32)
    out["bfc_t"] = (bfc - wfc_s.sum(axis=1)).reshape(12, 1).astype(np.float32)
    return out


def kernel(**inputs):
    if "nc" not in _NC_CACHE:
        _NC_CACHE["nc"] = _build_nc()
    nc = _NC_CACHE["nc"]
    const = _prep_const_inputs(inputs)
    x = np.asarray(inputs["x"], np.float32)
    in_maps = []
    for c in range(NCORES):
        m = dict(const)
        m["xs"] = np.ascontiguousarray(x[c * BPC:(c + 1) * BPC])
        in_maps.append(m)
    res = run_bass_kernel_spmd(nc, in_maps, core_ids=list(range(NCORES)))
    return np.concatenate([r["out"] for r in res.results], axis=0)
